# revision 14
# baseline (speedup 1.0000x reference)
"""Exact attention (B=2, N=2048, H=16, D=64, fp32) on 8 Trainium2 NeuronCores.

Sharding: the 32 (batch, head) pairs are split across 8 cores, 4 heads per
core. Each core computes full (non-causal, unscaled) attention for its heads.

Per-core schedule: an ACT-bound software pipeline. The exp stream (128
instructions of [128, 1024], ~0.98us each, ~125us total) is the hard floor;
everything else is scheduled to hide underneath it.

  - 64 "slots", one per (pair, n-half, m-block): slot s emits
      S^T(s) on PE   (4 f32r matmuls [64x128 stationary, 64x512 moving])
      exp(s) on ACT  (one [128, 1024] PSUM->SBUF instruction, bf16 out)
      O^T(s-2) on PE (2 bf16 matmuls [128x65 stationary, 128x1024 moving])
    The 2-slot O deferral keeps PE's in-order queue from ever waiting on
    ACT: by the time O(s-2) issues, exp(s-2) finished a slot ago.
  - Q/K are PE-transposed to [d, n] head-pair-packed layout in batches of 8
    tiles through a PSUM staging tile + one [128, 1024] DVE copy. Pair-0
    batches borrow the O-accumulator PSUM banks before the first O matmul;
    pair-1 batches trickle through the S-tile PSUM pool mid-phase.
  - Finalize per (pair, nh): DVE flushes O^T [65,1024] to SBUF, PE
    transposes 128-col chunks into a [128, 8, 128] PSUM batch (reusing the
    just-freed O banks), DVE reciprocal+scale, DMA out. Interleaved with the
    next phase's slots.

Numerics: S matmuls in float32r (11-bit mantissa, full PE rate); P = exp(S)
stored bf16 (P in [0, e^~45], rel err ~0.2%); O accumulated in fp32 PSUM.
"""

import os
import sys

os.environ.setdefault("MYCRO_LOCAL_CACHE", "1")
sys.path.insert(0, "/opt/trn_rl_repo")

import numpy as np

import concourse.bacc as bacc
import concourse.mybir as mybir
import concourse.tile as tile
from concourse.bass_utils import run_bass_kernel_spmd
from concourse.masks import make_identity

f32 = mybir.dt.float32
f32r = mybir.dt.float32r
bf16 = mybir.dt.bfloat16

B, N, H, D = 2, 2048, 16, 64
HEADS_PER_CORE = 4
N_CORES = 8
NH = 1024          # n-half width (exp tile free dim)
N_MB = N // 128    # 16 m-blocks of 128 keys
DV = D + 1         # V plus ones column
N_SLOT = 64        # 2 pairs x 2 n-halves x 16 m-blocks
DEFER = 2          # O matmuls run this many slots behind S/exp


def emit_body(nc, q, k, v, out, pools):
    (const, sgp, vsp, qkt, vt_p, spool, ppool, opool, otflush, finsb) = pools
    identity = const["identity"]
    ones = const["ones"]

    state = {}  # per-pair staging/persistent tiles

    def stage_pair(pair):
        """Issue staging DMAs for a pair's Q, K, V.

        Chunk order K01 Q01 K23 Q23 so the first transpose batches (K tiles
        0-7, Q tiles 0-7) have their data as early as possible. sg tiles are
        f32r via bitcast so the PE transposes run at 1.5 cy/row.
        """
        h0, h1 = 2 * pair, 2 * pair + 1
        st = {}
        for src, nm in ((k, "k"), (q, "q")):
            st[nm] = sgp.tile([128, N // 128, 128], f32r,
                              name=f"sg_{nm}_{pair}", tag=f"sg_{nm}")
        for gg in range(2):
            for src, nm in ((k, "k"), (q, "q")):
                for g in (2 * gg, 2 * gg + 1):
                    gt = slice(g * 4, (g + 1) * 4)
                    gr = slice(g * 512, (g + 1) * 512)
                    for hi, hh in ((0, h0), (1, h1)):
                        nc.sync.dma_start(
                            out=st[nm][:, gt, 64 * hi:64 * hi + 64],
                            in_=src.bitcast(f32r)[hh, gr, :].rearrange(
                                "(t p) d -> p t d", p=128))
        st["v"] = []
        for i, hh in enumerate((h0, h1)):
            vs = vsp.tile([128, N_MB, 64], f32, name=f"vs_{hh}", tag=f"vs{i}")
            nc.sync.dma_start(
                out=vs, in_=v[hh].rearrange("(mb p) d -> p mb d", p=128))
            st["v"].append(vs)
        qt = qkt.tile([128, N], f32r, name=f"qt_{pair}", tag="qt")
        kt = qkt.tile([128, N], f32r, name=f"kt_{pair}", tag="kt")
        st["qt"], st["kt"] = qt, kt
        state[pair] = st

    def vt_convert(pair):
        """V f32 staging -> bf16 [128, mb, 65] with ones column."""
        vts = []
        for i in range(2):
            vt = vt_p.tile([128, N_MB, DV], bf16,
                           name=f"vt_{pair}_{i}", tag=f"vt{2 * pair + i}")
            nc.vector.tensor_copy(vt[:, :, 0:64], state[pair]["v"][i])
            nc.vector.tensor_copy(vt[:, :, 64:65], ones)
            vts.append(vt)
        state[pair]["vt"] = vts

    def t_batch(pair, nm, b, pool, tag):
        """Transpose 8 staged [128,128] tiles into qt/kt cols b*1024..+1024.

        One PSUM batch tile + one [128, 1024] DVE copy (f32 -> f32r round).
        """
        sg = state[pair][nm]
        dst = state[pair]["qt" if nm == "q" else "kt"]
        tb = pool.tile([128, 8, 128], f32r, name=f"tb_{pair}_{nm}_{b}", tag=tag)
        for idx in range(8):
            t = b * 8 + idx
            nc.tensor.transpose(tb[:, idx, :], sg[:, t, :], const["identity_r"])
        nc.vector.tensor_copy(
            dst[:, b * 1024:(b + 1) * 1024],
            tb.rearrange("p a b -> p (a b)"))

    def emit_S(pair, nh, mb, s):
        qt, kt = state[pair]["qt"], state[pair]["kt"]
        msl = slice(mb * 128, (mb + 1) * 128)
        sps = []
        for i, plo in ((0, 0), (1, 64)):
            sp = spool.tile([128, NH], f32, name=f"sp_{s}_{i}", tag="s")
            for j in range(2):
                jsl = slice(nh * NH + j * 512, nh * NH + (j + 1) * 512)
                nc.tensor.matmul(
                    out=sp[:, j * 512:(j + 1) * 512],
                    lhsT=kt[plo:plo + 64, msl],
                    rhs=qt[plo:plo + 64, jsl], start=True, stop=True)
            sps.append(sp)
        return sps

    def emit_exp(sps, s):
        pts = []
        for i in range(2):
            pt = ppool.tile([128, NH], bf16, name=f"pt_{s}_{i}", tag="p")
            nc.scalar.activation(
                out=pt, in_=sps[i], func=mybir.ActivationFunctionType.Exp)
            pts.append(pt)
        return pts

    def emit_O(pair, nh, mb, pts, oaccs):
        # j-sliced so each matmul's PSUM write stays inside one bank
        for i in range(2):
            for j in range(2):
                jsl = slice(j * 512, (j + 1) * 512)
                nc.tensor.matmul(
                    out=oaccs[i][:, jsl], lhsT=state[pair]["vt"][i][:, mb, :],
                    rhs=pts[i][:, jsl], start=(mb == 0), stop=(mb == N_MB - 1))

    def emit_flush(phase, oaccs):
        pair, nh = divmod(phase, 2)
        otss = []
        for i in range(2):
            ots = otflush.tile([65, NH], f32,
                               name=f"ots_{phase}_{i}", tag=f"ots{i}")
            nc.vector.tensor_copy(ots, oaccs[i])
            otss.append(ots)
        return otss

    def emit_fins(phase, otss):
        """Transpose + normalize + DMA out for one (pair, nh)."""
        pair, nh = divmod(phase, 2)
        for i in range(2):
            hh = 2 * pair + i
            fb = opool.tile([128, 8, 128], f32,
                            name=f"fb_{phase}_{i}", tag=f"o{i}")
            for c in range(8):
                nc.tensor.transpose(
                    fb[:, c, 0:65], otss[i][:, c * 128:(c + 1) * 128],
                    identity[0:65, 0:65])
            ostage = finsb.tile([128, 8, 64], f32,
                                name=f"ostage_{phase}_{i}", tag=f"ostage{i}")
            rcp = finsb.tile([128, 8, 1], f32,
                             name=f"rcp_{phase}_{i}", tag=f"rcp{i}")
            nc.vector.reciprocal(rcp, fb[:, :, 64:65])
            nc.vector.tensor_mul(ostage, fb[:, :, 0:64],
                                 rcp.broadcast_to([128, 8, 64]))
            nc.sync.dma_start(
                out=out[hh].rearrange("(cc p) d -> p cc d", p=128)[
                    :, nh * 8:(nh + 1) * 8, :],
                in_=ostage)

    # ---- schedule ----
    stage_pair(0)
    # pair-0 transpose batches borrow the o-tags before the first O alloc
    t_batch(0, "k", 0, opool, "o0")
    t_batch(0, "q", 0, opool, "o1")
    t_batch(0, "k", 1, opool, "o0")
    t_batch(0, "q", 1, opool, "o1")
    vt_convert(0)

    # pair-1 transpose batches trickle through spool mid-phase, away from
    # the phase-boundary congestion at s=32-35
    p1_tjobs = {12: ("k", 0), 14: ("q", 0), 24: ("k", 1), 26: ("q", 1)}

    pend = {}          # slot -> (sps tiles awaiting exp->O)
    oaccs_by_phase = {}
    otss_by_phase = {}

    for s in range(N_SLOT + DEFER):
        if s < N_SLOT:
            phase, mb = divmod(s, 16)
            pair, nh = divmod(phase, 2)
            if s == 8:
                stage_pair(1)
            if s == 22:
                vt_convert(1)
            if s in p1_tjobs:
                nm, b = p1_tjobs[s]
                t_batch(1, nm, b, spool, "s")
            sps = emit_S(pair, nh, mb, s)
            pend[s] = emit_exp(sps, s)
        # fins for phase P right after its flush — BEFORE this slot's O
        # block, so fb(P) precedes oacc(P+1) in the o-tag rotation
        if s >= 18 and (s - 18) % 16 == 0:
            emit_fins((s - 18) // 16, otss_by_phase.pop((s - 18) // 16))
        if s >= DEFER:
            s2 = s - DEFER
            phase2, mb2 = divmod(s2, 16)
            pair2, nh2 = divmod(phase2, 2)
            if mb2 == 0:
                oaccs_by_phase[phase2] = [
                    opool.tile([65, NH], f32, name=f"o_{phase2}_{i}",
                               tag=f"o{i}")
                    for i in range(2)
                ]
            emit_O(pair2, nh2, mb2, pend.pop(s2), oaccs_by_phase[phase2])
            if mb2 == N_MB - 1:
                otss_by_phase[phase2] = emit_flush(
                    phase2, oaccs_by_phase.pop(phase2))
    emit_fins(3, otss_by_phase.pop(3))


def build(repeat=1):
    nc = bacc.Bacc("TRN2", target_bir_lowering=False, debug=False)
    q = nc.dram_tensor("q", [HEADS_PER_CORE, N, D], f32, kind="ExternalInput").ap()
    k = nc.dram_tensor("k", [HEADS_PER_CORE, N, D], f32, kind="ExternalInput").ap()
    v = nc.dram_tensor("v", [HEADS_PER_CORE, N, D], f32, kind="ExternalInput").ap()
    out = nc.dram_tensor("out", [HEADS_PER_CORE, N, D], f32, kind="ExternalOutput").ap()

    from contextlib import ExitStack
    with tile.TileContext(nc) as tc, ExitStack() as ctx:
        const_pool = ctx.enter_context(tc.tile_pool(name="const", bufs=1))
        identity = const_pool.tile([128, 128], f32, name="identity")
        make_identity(nc, identity)
        # f32r copy via DVE: a "rounding producer" the BIR verifier accepts
        # as an f32r-matmult operand source
        identity_r = const_pool.tile([128, 128], f32r, name="identity_r")
        nc.vector.tensor_copy(identity_r, identity)
        ones = const_pool.tile([128, N_MB, 1], f32, name="ones")
        nc.vector.memset(ones, 1.0)

        sgp = ctx.enter_context(tc.tile_pool(name="sgp", bufs=2))
        vsp = ctx.enter_context(tc.tile_pool(name="vsp", bufs=2))
        qkt = ctx.enter_context(tc.tile_pool(name="qkt", bufs=2))
        vt_p = ctx.enter_context(tc.tile_pool(name="vt", bufs=1))
        spool = ctx.enter_context(tc.tile_pool(name="spool", bufs=2, space="PSUM"))
        ppool = ctx.enter_context(tc.tile_pool(name="ppool", bufs=8))
        opool = ctx.enter_context(tc.tile_pool(name="opool", bufs=1, space="PSUM"))
        otflush = ctx.enter_context(tc.tile_pool(name="otflush", bufs=2))
        finsb = ctx.enter_context(tc.tile_pool(name="finsb", bufs=2))

        pools = ({"identity": identity, "identity_r": identity_r,
                  "ones": ones}, sgp, vsp, qkt, vt_p,
                 spool, ppool, opool, otflush, finsb)

        if repeat == 1:
            emit_body(nc, q, k, v, out, pools)
        else:
            # hint_engines: the body far exceeds one IRAM block per engine,
            # so arm the back-edge branch prefetch to avoid ~4us I$-miss
            # stalls per iteration in the timing loop.
            with tc.For_i(0, repeat, 1, hint_engines=(
                    mybir.EngineType.PE, mybir.EngineType.Activation,
                    mybir.EngineType.DVE, mybir.EngineType.SP)):
                emit_body(nc, q, k, v, out, pools)

    nc.compile()
    return nc


_NC_CACHE = {}


def _get_nc(repeat=1):
    if repeat not in _NC_CACHE:
        _NC_CACHE[repeat] = build(repeat)
    return _NC_CACHE[repeat]


def run_sharded(query, key, value, repeat=1, **spmd_kwargs):
    """query/key/value: [B, N, H, D] fp32 -> out [B, H, N, D] fp32."""
    nc = _get_nc(repeat)
    # [B, N, H, D] -> [B*H, N, D]
    qh = np.ascontiguousarray(np.transpose(query, (0, 2, 1, 3))).reshape(B * H, N, D)
    kh = np.ascontiguousarray(np.transpose(key, (0, 2, 1, 3))).reshape(B * H, N, D)
    vh = np.ascontiguousarray(np.transpose(value, (0, 2, 1, 3))).reshape(B * H, N, D)
    in_maps = [
        {
            "q": qh[c * HEADS_PER_CORE:(c + 1) * HEADS_PER_CORE],
            "k": kh[c * HEADS_PER_CORE:(c + 1) * HEADS_PER_CORE],
            "v": vh[c * HEADS_PER_CORE:(c + 1) * HEADS_PER_CORE],
        }
        for c in range(N_CORES)
    ]
    res = run_bass_kernel_spmd(nc, in_maps, core_ids=list(range(N_CORES)),
                               **spmd_kwargs)
    outs = np.stack([res.results[c]["out"] for c in range(N_CORES)])  # [8, 4, N, D]
    return outs.reshape(B, H, N, D)


def kernel(query, key, value):
    query = np.asarray(query, dtype=np.float32)
    key = np.asarray(key, dtype=np.float32)
    value = np.asarray(value, dtype=np.float32)
    return run_sharded(query, key, value)


if __name__ == "__main__":
    rng = np.random.default_rng(0)
    q = rng.standard_normal((B, N, H, D), dtype=np.float32)
    k = rng.standard_normal((B, N, H, D), dtype=np.float32)
    v = rng.standard_normal((B, N, H, D), dtype=np.float32)
    o = kernel(q, k, v)
    print("out shape:", o.shape, o.dtype)


# revision 15
# speedup vs baseline: 1.0292x; 1.0292x over previous
"""Exact attention (B=2, N=2048, H=16, D=64, fp32) on 8 Trainium2 NeuronCores.

Sharding: the 32 (batch, head) pairs are split across 8 cores, 4 heads per
core. Each core computes full (non-causal, unscaled) attention for its heads.

Per-core schedule: an ACT-bound software pipeline. The exp stream (128
instructions of [128, 1024], ~0.98us each, ~125us total) is the hard floor;
everything else is scheduled to hide underneath it.

  - 64 "slots", one per (pair, n-half, m-block): slot s emits
      S^T(s) on PE   (4 f32r matmuls [64x128 stationary, 64x512 moving])
      exp(s) on ACT  (one [128, 1024] PSUM->SBUF instruction, bf16 out)
      O^T(s-2) on PE (2 bf16 matmuls [128x65 stationary, 128x1024 moving])
    The 2-slot O deferral keeps PE's in-order queue from ever waiting on
    ACT: by the time O(s-2) issues, exp(s-2) finished a slot ago.
  - Q/K are PE-transposed to [d, n] head-pair-packed layout in batches of 8
    tiles through a PSUM staging tile + one [128, 1024] DVE copy. Pair-0
    batches borrow the O-accumulator PSUM banks before the first O matmul;
    pair-1 batches trickle through the S-tile PSUM pool mid-phase.
  - Finalize per (pair, nh): DVE flushes O^T [65,1024] to SBUF, PE
    transposes 128-col chunks into a [128, 8, 128] PSUM batch (reusing the
    just-freed O banks), DVE reciprocal+scale, DMA out. Interleaved with the
    next phase's slots.

Numerics: S matmuls in float32r (11-bit mantissa, full PE rate); P = exp(S)
stored bf16 (P in [0, e^~45], rel err ~0.2%); O accumulated in fp32 PSUM.
"""

import os
import sys

os.environ.setdefault("MYCRO_LOCAL_CACHE", "1")
sys.path.insert(0, "/opt/trn_rl_repo")

import numpy as np

import concourse.bacc as bacc
import concourse.mybir as mybir
import concourse.tile as tile
from concourse.bass_utils import run_bass_kernel_spmd
from concourse.masks import make_identity

f32 = mybir.dt.float32
f32r = mybir.dt.float32r
bf16 = mybir.dt.bfloat16

B, N, H, D = 2, 2048, 16, 64
HEADS_PER_CORE = 4
N_CORES = 8
NH = 1024          # n-half width (exp tile free dim)
N_MB = N // 128    # 16 m-blocks of 128 keys
DV = D + 1         # V plus ones column
N_SLOT = 64        # 2 pairs x 2 n-halves x 16 m-blocks
DEFER = 2          # O matmuls run this many slots behind S/exp


def emit_body(nc, q, k, v, out, pools):
    (const, sgp, vsp, qkt, vt_p, spool, ppool, opool, otflush, finsb) = pools
    identity = const["identity"]
    ones = const["ones"]

    state = {}  # per-pair staging/persistent tiles

    def stage_pair(pair):
        """Issue staging DMAs for a pair's Q, K, V.

        Chunk order K01 Q01 K23 Q23 so the first transpose batches (K tiles
        0-7, Q tiles 0-7) have their data as early as possible. sg tiles are
        f32r via bitcast so the PE transposes run at 1.5 cy/row.
        """
        h0, h1 = 2 * pair, 2 * pair + 1
        st = {}
        for src, nm in ((k, "k"), (q, "q")):
            st[nm] = sgp.tile([128, N // 128, 128], f32,
                              name=f"sg_{nm}_{pair}", tag=f"sg_{nm}")
        for gg in range(2):
            for src, nm in ((k, "k"), (q, "q")):
                for g in (2 * gg, 2 * gg + 1):
                    gt = slice(g * 4, (g + 1) * 4)
                    gr = slice(g * 512, (g + 1) * 512)
                    for hi, hh in ((0, h0), (1, h1)):
                        nc.sync.dma_start(
                            out=st[nm][:, gt, 64 * hi:64 * hi + 64],
                            in_=src[hh, gr, :].rearrange(
                                "(t p) d -> p t d", p=128))
        st["v"] = []
        for i, hh in enumerate((h0, h1)):
            vs = vsp.tile([128, N_MB, 64], f32, name=f"vs_{hh}", tag=f"vs{i}")
            nc.sync.dma_start(
                out=vs, in_=v[hh].rearrange("(mb p) d -> p mb d", p=128))
            st["v"].append(vs)
        qt = qkt.tile([128, N], f32r, name=f"qt_{pair}", tag="qt")
        kt = qkt.tile([128, N], f32r, name=f"kt_{pair}", tag="kt")
        st["qt"], st["kt"] = qt, kt
        state[pair] = st

    def vt_convert(pair):
        """V f32 staging -> bf16 [128, mb, 65] with ones column."""
        vts = []
        for i in range(2):
            vt = vt_p.tile([128, N_MB, DV], bf16,
                           name=f"vt_{pair}_{i}", tag=f"vt{2 * pair + i}")
            nc.vector.tensor_copy(vt[:, :, 0:64], state[pair]["v"][i])
            nc.vector.tensor_copy(vt[:, :, 64:65], ones)
            vts.append(vt)
        state[pair]["vt"] = vts

    def t_batch(pair, nm, b, pool, tag):
        """Transpose 8 staged [128,128] tiles into qt/kt cols b*1024..+1024.

        One PSUM batch tile + one [128, 1024] DVE copy (f32 -> f32r round).
        """
        sg = state[pair][nm]
        dst = state[pair]["qt" if nm == "q" else "kt"]
        tb = pool.tile([128, 8, 128], f32, name=f"tb_{pair}_{nm}_{b}", tag=tag)
        for idx in range(8):
            t = b * 8 + idx
            nc.tensor.transpose(tb[:, idx, :], sg[:, t, :], identity)
        nc.vector.tensor_copy(
            dst[:, b * 1024:(b + 1) * 1024],
            tb.rearrange("p a b -> p (a b)"))

    def emit_S(pair, nh, mb, s):
        qt, kt = state[pair]["qt"], state[pair]["kt"]
        msl = slice(mb * 128, (mb + 1) * 128)
        sps = []
        for i, plo in ((0, 0), (1, 64)):
            sp = spool.tile([128, NH], f32, name=f"sp_{s}_{i}", tag="s")
            for j in range(2):
                jsl = slice(nh * NH + j * 512, nh * NH + (j + 1) * 512)
                nc.tensor.matmul(
                    out=sp[:, j * 512:(j + 1) * 512],
                    lhsT=kt[plo:plo + 64, msl],
                    rhs=qt[plo:plo + 64, jsl], start=True, stop=True)
            sps.append(sp)
        return sps

    def emit_exp(sps, s):
        pts = []
        for i in range(2):
            pt = ppool.tile([128, NH], bf16, name=f"pt_{s}_{i}", tag="p")
            nc.scalar.activation(
                out=pt, in_=sps[i], func=mybir.ActivationFunctionType.Exp)
            pts.append(pt)
        return pts

    def emit_O(pair, nh, mb, pts, oaccs):
        # j-sliced so each matmul's PSUM write stays inside one bank
        for i in range(2):
            for j in range(2):
                jsl = slice(j * 512, (j + 1) * 512)
                nc.tensor.matmul(
                    out=oaccs[i][:, jsl], lhsT=state[pair]["vt"][i][:, mb, :],
                    rhs=pts[i][:, jsl], start=(mb == 0), stop=(mb == N_MB - 1))

    def emit_flush(phase, oaccs):
        pair, nh = divmod(phase, 2)
        otss = []
        for i in range(2):
            ots = otflush.tile([65, NH], f32,
                               name=f"ots_{phase}_{i}", tag=f"ots{i}")
            nc.vector.tensor_copy(ots, oaccs[i])
            otss.append(ots)
        return otss

    def emit_fins(phase, otss):
        """Transpose + normalize + DMA out for one (pair, nh)."""
        pair, nh = divmod(phase, 2)
        for i in range(2):
            hh = 2 * pair + i
            fb = opool.tile([128, 8, 128], f32,
                            name=f"fb_{phase}_{i}", tag=f"o{i}")
            for c in range(8):
                nc.tensor.transpose(
                    fb[:, c, 0:65], otss[i][:, c * 128:(c + 1) * 128],
                    identity[0:65, 0:65])
            ostage = finsb.tile([128, 8, 64], f32,
                                name=f"ostage_{phase}_{i}", tag=f"ostage{i}")
            rcp = finsb.tile([128, 8, 1], f32,
                             name=f"rcp_{phase}_{i}", tag=f"rcp{i}")
            nc.vector.reciprocal(rcp, fb[:, :, 64:65])
            nc.vector.tensor_mul(ostage, fb[:, :, 0:64],
                                 rcp.broadcast_to([128, 8, 64]))
            nc.sync.dma_start(
                out=out[hh].rearrange("(cc p) d -> p cc d", p=128)[
                    :, nh * 8:(nh + 1) * 8, :],
                in_=ostage)

    # ---- schedule ----
    stage_pair(0)
    # pair-0 transpose batches borrow the o-tags before the first O alloc
    t_batch(0, "k", 0, opool, "o0")
    t_batch(0, "q", 0, opool, "o1")
    t_batch(0, "k", 1, opool, "o0")
    t_batch(0, "q", 1, opool, "o1")
    vt_convert(0)

    # pair-1 transpose batches trickle through spool mid-phase, away from
    # the phase-boundary congestion at s=32-35
    p1_tjobs = {12: ("k", 0), 14: ("q", 0), 24: ("k", 1), 26: ("q", 1)}

    pend = {}          # slot -> (sps tiles awaiting exp->O)
    oaccs_by_phase = {}
    otss_by_phase = {}

    for s in range(N_SLOT + DEFER):
        if s < N_SLOT:
            phase, mb = divmod(s, 16)
            pair, nh = divmod(phase, 2)
            if s == 8:
                stage_pair(1)
            if s == 22:
                vt_convert(1)
            if s in p1_tjobs:
                nm, b = p1_tjobs[s]
                t_batch(1, nm, b, spool, "s")
            sps = emit_S(pair, nh, mb, s)
            pend[s] = emit_exp(sps, s)
        # fins for phase P right after its flush — BEFORE this slot's O
        # block, so fb(P) precedes oacc(P+1) in the o-tag rotation
        if s >= 18 and (s - 18) % 16 == 0:
            emit_fins((s - 18) // 16, otss_by_phase.pop((s - 18) // 16))
        if s >= DEFER:
            s2 = s - DEFER
            phase2, mb2 = divmod(s2, 16)
            pair2, nh2 = divmod(phase2, 2)
            if mb2 == 0:
                oaccs_by_phase[phase2] = [
                    opool.tile([65, NH], f32, name=f"o_{phase2}_{i}",
                               tag=f"o{i}")
                    for i in range(2)
                ]
            emit_O(pair2, nh2, mb2, pend.pop(s2), oaccs_by_phase[phase2])
            if mb2 == N_MB - 1:
                otss_by_phase[phase2] = emit_flush(
                    phase2, oaccs_by_phase.pop(phase2))
    emit_fins(3, otss_by_phase.pop(3))


def build(repeat=1):
    nc = bacc.Bacc("TRN2", target_bir_lowering=False, debug=False)
    q = nc.dram_tensor("q", [HEADS_PER_CORE, N, D], f32, kind="ExternalInput").ap()
    k = nc.dram_tensor("k", [HEADS_PER_CORE, N, D], f32, kind="ExternalInput").ap()
    v = nc.dram_tensor("v", [HEADS_PER_CORE, N, D], f32, kind="ExternalInput").ap()
    out = nc.dram_tensor("out", [HEADS_PER_CORE, N, D], f32, kind="ExternalOutput").ap()

    from contextlib import ExitStack
    with tile.TileContext(nc) as tc, ExitStack() as ctx:
        const_pool = ctx.enter_context(tc.tile_pool(name="const", bufs=1))
        identity = const_pool.tile([128, 128], f32, name="identity")
        make_identity(nc, identity)
        # f32r copy via DVE: a "rounding producer" the BIR verifier accepts
        # as an f32r-matmult operand source
        identity_r = const_pool.tile([128, 128], f32r, name="identity_r")
        nc.vector.tensor_copy(identity_r, identity)
        ones = const_pool.tile([128, N_MB, 1], f32, name="ones")
        nc.vector.memset(ones, 1.0)

        sgp = ctx.enter_context(tc.tile_pool(name="sgp", bufs=2))
        vsp = ctx.enter_context(tc.tile_pool(name="vsp", bufs=2))
        qkt = ctx.enter_context(tc.tile_pool(name="qkt", bufs=2))
        vt_p = ctx.enter_context(tc.tile_pool(name="vt", bufs=1))
        spool = ctx.enter_context(tc.tile_pool(name="spool", bufs=2, space="PSUM"))
        ppool = ctx.enter_context(tc.tile_pool(name="ppool", bufs=8))
        opool = ctx.enter_context(tc.tile_pool(name="opool", bufs=1, space="PSUM"))
        otflush = ctx.enter_context(tc.tile_pool(name="otflush", bufs=2))
        finsb = ctx.enter_context(tc.tile_pool(name="finsb", bufs=2))

        pools = ({"identity": identity, "identity_r": identity_r,
                  "ones": ones}, sgp, vsp, qkt, vt_p,
                 spool, ppool, opool, otflush, finsb)

        if repeat == 1:
            emit_body(nc, q, k, v, out, pools)
        else:
            # hint_engines: the body far exceeds one IRAM block per engine,
            # so arm the back-edge branch prefetch to avoid ~4us I$-miss
            # stalls per iteration in the timing loop.
            with tc.For_i(0, repeat, 1, hint_engines=(
                    mybir.EngineType.PE, mybir.EngineType.Activation,
                    mybir.EngineType.DVE, mybir.EngineType.SP)):
                emit_body(nc, q, k, v, out, pools)

    nc.compile()
    return nc


_NC_CACHE = {}


def _get_nc(repeat=1):
    if repeat not in _NC_CACHE:
        _NC_CACHE[repeat] = build(repeat)
    return _NC_CACHE[repeat]


def run_sharded(query, key, value, repeat=1, **spmd_kwargs):
    """query/key/value: [B, N, H, D] fp32 -> out [B, H, N, D] fp32."""
    nc = _get_nc(repeat)
    # [B, N, H, D] -> [B*H, N, D]
    qh = np.ascontiguousarray(np.transpose(query, (0, 2, 1, 3))).reshape(B * H, N, D)
    kh = np.ascontiguousarray(np.transpose(key, (0, 2, 1, 3))).reshape(B * H, N, D)
    vh = np.ascontiguousarray(np.transpose(value, (0, 2, 1, 3))).reshape(B * H, N, D)
    in_maps = [
        {
            "q": qh[c * HEADS_PER_CORE:(c + 1) * HEADS_PER_CORE],
            "k": kh[c * HEADS_PER_CORE:(c + 1) * HEADS_PER_CORE],
            "v": vh[c * HEADS_PER_CORE:(c + 1) * HEADS_PER_CORE],
        }
        for c in range(N_CORES)
    ]
    res = run_bass_kernel_spmd(nc, in_maps, core_ids=list(range(N_CORES)),
                               **spmd_kwargs)
    outs = np.stack([res.results[c]["out"] for c in range(N_CORES)])  # [8, 4, N, D]
    return outs.reshape(B, H, N, D)


def kernel(query, key, value):
    query = np.asarray(query, dtype=np.float32)
    key = np.asarray(key, dtype=np.float32)
    value = np.asarray(value, dtype=np.float32)
    return run_sharded(query, key, value)


if __name__ == "__main__":
    rng = np.random.default_rng(0)
    q = rng.standard_normal((B, N, H, D), dtype=np.float32)
    k = rng.standard_normal((B, N, H, D), dtype=np.float32)
    v = rng.standard_normal((B, N, H, D), dtype=np.float32)
    o = kernel(q, k, v)
    print("out shape:", o.shape, o.dtype)


# revision 16
# speedup vs baseline: 1.0371x; 1.0076x over previous
"""Exact attention (B=2, N=2048, H=16, D=64, fp32) on 8 Trainium2 NeuronCores.

Sharding: the 32 (batch, head) pairs are split across 8 cores, 4 heads per
core. Each core computes full (non-causal, unscaled) attention for its heads.

Per-core schedule: an ACT-bound software pipeline. The exp stream (128
instructions of [128, 1024], ~0.98us each, ~125us total) is the hard floor;
everything else is scheduled to hide underneath it.

  - 64 "slots", one per (pair, n-half, m-block): slot s emits
      S^T(s) on PE   (4 f32r matmuls [64x128 stationary, 64x512 moving])
      exp(s) on ACT  (one [128, 1024] PSUM->SBUF instruction, bf16 out)
      O^T(s-2) on PE (2 bf16 matmuls [128x65 stationary, 128x1024 moving])
    The 2-slot O deferral keeps PE's in-order queue from ever waiting on
    ACT: by the time O(s-2) issues, exp(s-2) finished a slot ago.
  - Q/K are PE-transposed to [d, n] head-pair-packed layout in batches of 8
    tiles through a PSUM staging tile + one [128, 1024] DVE copy. Pair-0
    batches borrow the O-accumulator PSUM banks before the first O matmul;
    pair-1 batches trickle through the S-tile PSUM pool mid-phase.
  - Finalize per (pair, nh): DVE flushes O^T [65,1024] to SBUF, PE
    transposes 128-col chunks into a [128, 8, 128] PSUM batch (reusing the
    just-freed O banks), DVE reciprocal+scale, DMA out. Interleaved with the
    next phase's slots.

Numerics: S matmuls in float32r (11-bit mantissa, full PE rate); P = exp(S)
stored bf16 (P in [0, e^~45], rel err ~0.2%); O accumulated in fp32 PSUM.
"""

import os
import sys

os.environ.setdefault("MYCRO_LOCAL_CACHE", "1")
sys.path.insert(0, "/opt/trn_rl_repo")

import numpy as np

import concourse.bacc as bacc
import concourse.mybir as mybir
import concourse.tile as tile
from concourse.bass_utils import run_bass_kernel_spmd
from concourse.masks import make_identity

f32 = mybir.dt.float32
f32r = mybir.dt.float32r
bf16 = mybir.dt.bfloat16

B, N, H, D = 2, 2048, 16, 64
HEADS_PER_CORE = 4
N_CORES = 8
NH = 1024          # n-half width (exp tile free dim)
N_MB = N // 128    # 16 m-blocks of 128 keys
DV = D + 1         # V plus ones column
N_SLOT = 64        # 2 pairs x 2 n-halves x 16 m-blocks
DEFER = 2          # O matmuls run this many slots behind S/exp


def emit_body(nc, q, k, v, out, pools):
    (const, sgp, vsp, qkt, vt_p, spool, ppool, opool, otflush, finsb) = pools
    identity = const["identity"]
    ones = const["ones"]

    state = {}  # per-pair staging/persistent tiles

    def stage_pair(pair):
        """Issue staging DMAs for a pair's Q, K, V.

        Chunk order K01 Q01 K23 Q23 so the first transpose batches (K tiles
        0-7, Q tiles 0-7) have their data as early as possible. sg tiles are
        f32r via bitcast so the PE transposes run at 1.5 cy/row.
        """
        h0, h1 = 2 * pair, 2 * pair + 1
        st = {}
        for src, nm in ((k, "k"), (q, "q")):
            st[nm] = sgp.tile([128, N // 128, 128], f32,
                              name=f"sg_{nm}_{pair}", tag=f"sg_{nm}")
        for gg in range(2):
            for src, nm in ((k, "k"), (q, "q")):
                for g in (2 * gg, 2 * gg + 1):
                    gt = slice(g * 4, (g + 1) * 4)
                    gr = slice(g * 512, (g + 1) * 512)
                    for hi, hh in ((0, h0), (1, h1)):
                        nc.sync.dma_start(
                            out=st[nm][:, gt, 64 * hi:64 * hi + 64],
                            in_=src[hh, gr, :].rearrange(
                                "(t p) d -> p t d", p=128))
        st["v"] = []
        for i, hh in enumerate((h0, h1)):
            vs = vsp.tile([128, N_MB, 64], f32, name=f"vs_{hh}", tag=f"vs{i}")
            nc.sync.dma_start(
                out=vs, in_=v[hh].rearrange("(mb p) d -> p mb d", p=128))
            st["v"].append(vs)
        qt = qkt.tile([128, N], f32r, name=f"qt_{pair}", tag="qt")
        kt = qkt.tile([128, N], f32r, name=f"kt_{pair}", tag="kt")
        st["qt"], st["kt"] = qt, kt
        state[pair] = st

    def vt_convert(pair):
        """V f32 staging -> bf16 [128, mb, 65] with ones column."""
        vts = []
        for i in range(2):
            vt = vt_p.tile([128, N_MB, DV], bf16,
                           name=f"vt_{pair}_{i}", tag=f"vt{2 * pair + i}")
            nc.vector.tensor_copy(vt[:, :, 0:64], state[pair]["v"][i])
            nc.vector.tensor_copy(vt[:, :, 64:65], ones)
            vts.append(vt)
        state[pair]["vt"] = vts

    def t_batch(pair, nm, b, pool, tag):
        """Transpose 8 staged [128,128] tiles into qt/kt cols b*1024..+1024.

        One PSUM batch tile + one [128, 1024] DVE copy (f32 -> f32r round).
        """
        sg = state[pair][nm]
        dst = state[pair]["qt" if nm == "q" else "kt"]
        tb = pool.tile([128, 8, 128], f32, name=f"tb_{pair}_{nm}_{b}", tag=tag)
        for idx in range(8):
            t = b * 8 + idx
            nc.tensor.transpose(tb[:, idx, :], sg[:, t, :], identity)
        nc.vector.tensor_copy(
            dst[:, b * 1024:(b + 1) * 1024],
            tb.rearrange("p a b -> p (a b)"))

    def emit_S(pair, nh, mb, s):
        qt, kt = state[pair]["qt"], state[pair]["kt"]
        msl = slice(mb * 128, (mb + 1) * 128)
        sps = []
        for i, plo in ((0, 0), (1, 64)):
            sp = spool.tile([128, NH], f32, name=f"sp_{s}_{i}", tag="s")
            for j in range(2):
                jsl = slice(nh * NH + j * 512, nh * NH + (j + 1) * 512)
                nc.tensor.matmul(
                    out=sp[:, j * 512:(j + 1) * 512],
                    lhsT=kt[plo:plo + 64, msl],
                    rhs=qt[plo:plo + 64, jsl], start=True, stop=True)
            sps.append(sp)
        return sps

    def emit_exp(sps, s):
        pts = []
        for i in range(2):
            pt = ppool.tile([128, NH], bf16, name=f"pt_{s}_{i}", tag="p")
            nc.scalar.activation(
                out=pt, in_=sps[i], func=mybir.ActivationFunctionType.Exp)
            pts.append(pt)
        return pts

    def emit_O(pair, nh, mb, pts, oaccs):
        # j-sliced so each matmul's PSUM write stays inside one bank
        for i in range(2):
            for j in range(2):
                jsl = slice(j * 512, (j + 1) * 512)
                nc.tensor.matmul(
                    out=oaccs[i][:, jsl], lhsT=state[pair]["vt"][i][:, mb, :],
                    rhs=pts[i][:, jsl], start=(mb == 0), stop=(mb == N_MB - 1))

    def emit_flush(phase, oaccs):
        pair, nh = divmod(phase, 2)
        otss = []
        for i in range(2):
            ots = otflush.tile([65, NH], f32,
                               name=f"ots_{phase}_{i}", tag=f"ots{i}")
            nc.vector.tensor_copy(ots, oaccs[i])
            otss.append(ots)
        return otss

    def emit_fins(phase, otss):
        """Transpose + normalize + DMA out for one (pair, nh)."""
        pair, nh = divmod(phase, 2)
        for i in range(2):
            hh = 2 * pair + i
            fb = opool.tile([128, 8, 128], f32,
                            name=f"fb_{phase}_{i}", tag=f"o{i}")
            for c in range(8):
                nc.tensor.transpose(
                    fb[:, c, 0:65], otss[i][:, c * 128:(c + 1) * 128],
                    identity[0:65, 0:65])
            ostage = finsb.tile([128, 8, 64], f32,
                                name=f"ostage_{phase}_{i}", tag=f"ostage{i}")
            for c in range(8):
                rcp = finsb.tile([128, 1], f32,
                                 name=f"rcp_{phase}_{i}_{c}", tag=f"rcp{i}")
                nc.vector.reciprocal(rcp, fb[:, c, 64:65])
                nc.vector.tensor_scalar_mul(ostage[:, c, :], fb[:, c, 0:64], rcp)
            nc.sync.dma_start(
                out=out[hh].rearrange("(cc p) d -> p cc d", p=128)[
                    :, nh * 8:(nh + 1) * 8, :],
                in_=ostage)

    # ---- schedule ----
    stage_pair(0)
    # pair-0 transpose batches borrow the o-tags before the first O alloc
    t_batch(0, "k", 0, opool, "o0")
    t_batch(0, "q", 0, opool, "o1")
    t_batch(0, "k", 1, opool, "o0")
    t_batch(0, "q", 1, opool, "o1")
    vt_convert(0)

    # pair-1 transpose batches trickle through spool mid-phase, away from
    # the phase-boundary congestion at s=32-35
    p1_tjobs = {12: ("k", 0), 14: ("q", 0), 24: ("k", 1), 26: ("q", 1)}

    pend = {}          # slot -> (sps tiles awaiting exp->O)
    oaccs_by_phase = {}
    otss_by_phase = {}

    for s in range(N_SLOT + DEFER):
        if s < N_SLOT:
            phase, mb = divmod(s, 16)
            pair, nh = divmod(phase, 2)
            if s == 8:
                stage_pair(1)
            if s == 22:
                vt_convert(1)
            if s in p1_tjobs:
                nm, b = p1_tjobs[s]
                t_batch(1, nm, b, spool, "s")
            sps = emit_S(pair, nh, mb, s)
            pend[s] = emit_exp(sps, s)
        # fins for phase P right after its flush — BEFORE this slot's O
        # block, so fb(P) precedes oacc(P+1) in the o-tag rotation
        if s >= 18 and (s - 18) % 16 == 0:
            emit_fins((s - 18) // 16, otss_by_phase.pop((s - 18) // 16))
        if s >= DEFER:
            s2 = s - DEFER
            phase2, mb2 = divmod(s2, 16)
            pair2, nh2 = divmod(phase2, 2)
            if mb2 == 0:
                oaccs_by_phase[phase2] = [
                    opool.tile([65, NH], f32, name=f"o_{phase2}_{i}",
                               tag=f"o{i}")
                    for i in range(2)
                ]
            emit_O(pair2, nh2, mb2, pend.pop(s2), oaccs_by_phase[phase2])
            if mb2 == N_MB - 1:
                otss_by_phase[phase2] = emit_flush(
                    phase2, oaccs_by_phase.pop(phase2))
    emit_fins(3, otss_by_phase.pop(3))


def build(repeat=1):
    nc = bacc.Bacc("TRN2", target_bir_lowering=False, debug=False)
    q = nc.dram_tensor("q", [HEADS_PER_CORE, N, D], f32, kind="ExternalInput").ap()
    k = nc.dram_tensor("k", [HEADS_PER_CORE, N, D], f32, kind="ExternalInput").ap()
    v = nc.dram_tensor("v", [HEADS_PER_CORE, N, D], f32, kind="ExternalInput").ap()
    out = nc.dram_tensor("out", [HEADS_PER_CORE, N, D], f32, kind="ExternalOutput").ap()

    from contextlib import ExitStack
    with tile.TileContext(nc) as tc, ExitStack() as ctx:
        const_pool = ctx.enter_context(tc.tile_pool(name="const", bufs=1))
        identity = const_pool.tile([128, 128], f32, name="identity")
        make_identity(nc, identity)
        # f32r copy via DVE: a "rounding producer" the BIR verifier accepts
        # as an f32r-matmult operand source
        identity_r = const_pool.tile([128, 128], f32r, name="identity_r")
        nc.vector.tensor_copy(identity_r, identity)
        ones = const_pool.tile([128, N_MB, 1], f32, name="ones")
        nc.vector.memset(ones, 1.0)

        sgp = ctx.enter_context(tc.tile_pool(name="sgp", bufs=2))
        vsp = ctx.enter_context(tc.tile_pool(name="vsp", bufs=2))
        qkt = ctx.enter_context(tc.tile_pool(name="qkt", bufs=2))
        vt_p = ctx.enter_context(tc.tile_pool(name="vt", bufs=1))
        spool = ctx.enter_context(tc.tile_pool(name="spool", bufs=2, space="PSUM"))
        ppool = ctx.enter_context(tc.tile_pool(name="ppool", bufs=8))
        opool = ctx.enter_context(tc.tile_pool(name="opool", bufs=1, space="PSUM"))
        otflush = ctx.enter_context(tc.tile_pool(name="otflush", bufs=2))
        finsb = ctx.enter_context(tc.tile_pool(name="finsb", bufs=2))

        pools = ({"identity": identity, "identity_r": identity_r,
                  "ones": ones}, sgp, vsp, qkt, vt_p,
                 spool, ppool, opool, otflush, finsb)

        if repeat == 1:
            emit_body(nc, q, k, v, out, pools)
        else:
            # hint_engines: the body far exceeds one IRAM block per engine,
            # so arm the back-edge branch prefetch to avoid ~4us I$-miss
            # stalls per iteration in the timing loop.
            with tc.For_i(0, repeat, 1, hint_engines=(
                    mybir.EngineType.PE, mybir.EngineType.Activation,
                    mybir.EngineType.DVE, mybir.EngineType.SP)):
                emit_body(nc, q, k, v, out, pools)

    nc.compile()
    return nc


_NC_CACHE = {}


def _get_nc(repeat=1):
    if repeat not in _NC_CACHE:
        _NC_CACHE[repeat] = build(repeat)
    return _NC_CACHE[repeat]


def run_sharded(query, key, value, repeat=1, **spmd_kwargs):
    """query/key/value: [B, N, H, D] fp32 -> out [B, H, N, D] fp32."""
    nc = _get_nc(repeat)
    # [B, N, H, D] -> [B*H, N, D]
    qh = np.ascontiguousarray(np.transpose(query, (0, 2, 1, 3))).reshape(B * H, N, D)
    kh = np.ascontiguousarray(np.transpose(key, (0, 2, 1, 3))).reshape(B * H, N, D)
    vh = np.ascontiguousarray(np.transpose(value, (0, 2, 1, 3))).reshape(B * H, N, D)
    in_maps = [
        {
            "q": qh[c * HEADS_PER_CORE:(c + 1) * HEADS_PER_CORE],
            "k": kh[c * HEADS_PER_CORE:(c + 1) * HEADS_PER_CORE],
            "v": vh[c * HEADS_PER_CORE:(c + 1) * HEADS_PER_CORE],
        }
        for c in range(N_CORES)
    ]
    res = run_bass_kernel_spmd(nc, in_maps, core_ids=list(range(N_CORES)),
                               **spmd_kwargs)
    outs = np.stack([res.results[c]["out"] for c in range(N_CORES)])  # [8, 4, N, D]
    return outs.reshape(B, H, N, D)


def kernel(query, key, value):
    query = np.asarray(query, dtype=np.float32)
    key = np.asarray(key, dtype=np.float32)
    value = np.asarray(value, dtype=np.float32)
    return run_sharded(query, key, value)


if __name__ == "__main__":
    rng = np.random.default_rng(0)
    q = rng.standard_normal((B, N, H, D), dtype=np.float32)
    k = rng.standard_normal((B, N, H, D), dtype=np.float32)
    v = rng.standard_normal((B, N, H, D), dtype=np.float32)
    o = kernel(q, k, v)
    print("out shape:", o.shape, o.dtype)
